# revision 11
# baseline (speedup 1.0000x reference)
import math
import sys

import numpy as np

sys.path.insert(0, "/opt/trn_rl_repo")

import concourse.bass as bass
import concourse.mybir as mybir
import concourse.tile as tile
from concourse import bacc
from concourse.bass_utils import run_bass_kernel_spmd
from concourse.masks import make_identity

P = 128
B, S, DM, NH, L, DFF, KD, VOCAB = 4, 1024, 512, 8, 6, 2048, 512, 522
DH = DM
DO = DM // P
VP = 640
VC = VP // P
TL = B * P
TH = TL // 2
FO = DFF // P
NG = S // 512
NCORES = 8
EPS = 1e-5
F32 = mybir.dt.float32
F32R = mybir.dt.float32r
BF16 = mybir.dt.bfloat16
NEG = -30000.0
AF = mybir.ActivationFunctionType
OP = mybir.AluOpType


def _ln(nc, sb, ps, xT, g_ap, b_ap, ones128_r, eps_t, name):
    T = xT.shape[2]
    sq = sb.tile([P, DO, T], F32R, tag="ln_t", bufs=2, name=f"{name}_sq")
    nc.vector.tensor_tensor(sq[:], xT[:], xT[:], OP.mult)
    s1 = ps.tile([P, T], F32, tag="ln_ps", bufs=1, name=f"{name}_s1")
    s2 = ps.tile([P, T], F32, tag="ln_ps", bufs=1, name=f"{name}_s2")
    for c in range(DO):
        nc.tensor.matmul(s1[:], ones128_r[:], xT[:, c, :], start=(c == 0), stop=(c == DO - 1))
    for c in range(DO):
        nc.tensor.matmul(s2[:], ones128_r[:], sq[:, c, :], start=(c == 0), stop=(c == DO - 1))
    row = lambda nm: sb.tile([P, T], F32, tag="ln_row", bufs=7, name=f"{name}_{nm}")
    mean = row("mean")
    nc.vector.tensor_scalar(mean[:], s1[:], 1.0 / DM, None, OP.mult)
    m2 = row("m2")
    nc.vector.tensor_tensor(m2[:], mean[:], mean[:], OP.mult)
    ex2 = row("ex2")
    nc.vector.tensor_scalar(ex2[:], s2[:], 1.0 / DM, None, OP.mult)
    var = row("var")
    nc.vector.tensor_tensor(var[:], ex2[:], m2[:], OP.subtract)
    std = row("std")
    nc.scalar.activation(std[:], var[:], AF.Sqrt, bias=eps_t[:], scale=1.0)
    rstd = row("rstd")
    nc.vector.reciprocal(rstd[:], std[:])
    mrs = row("mrs")
    nc.vector.tensor_tensor(mrs[:], mean[:], rstd[:], OP.mult)
    out = sb.tile([P, DO, T], F32R, tag="ln_out", bufs=4, name=f"{name}_out")
    u = sb.tile([P, DO, T], F32, tag="ln_t", bufs=2, name=f"{name}_u")
    for c in range(DO):
        nc.vector.tensor_tensor(u[:, c, :], xT[:, c, :], rstd[:], OP.mult)
        nc.vector.tensor_tensor(u[:, c, :], u[:, c, :], mrs[:], OP.subtract)
        nc.vector.tensor_scalar(out[:, c, :], u[:, c, :], g_ap[:, c:c + 1],
                                b_ap[:, c:c + 1], OP.mult, OP.add)
    return out


def build_nc():
    nc = bacc.Bacc(None, target_bir_lowering=False, debug=False)

    ein = lambda nm, shp, dt=F32: nc.dram_tensor(nm, shp, dt, kind="ExternalInput")
    x_f = ein("x_f", [1, TL])
    tok_emb = ein("tok_emb", [P, VC, DM], F32R)
    pos_emb = ein("pos_emb", [P, 1, DM], F32R)
    ln_in_g, ln_in_b = ein("ln_in_g", [P, DO]), ein("ln_in_b", [P, DO])
    Wq, Wk, Wv = (ein(n, [L, P, DO, DH], F32R) for n in ("Wq", "Wk", "Wv"))
    Wo = ein("Wo", [L, P, DO, DM], F32R)
    W1 = ein("W1", [L, P, DO, DFF], F32R)
    W2 = ein("W2", [L, P, FO, DM], F32R)
    bq, bk = ein("bq", [L, P, DO]), ein("bk", [L, P, DO])
    bv = ein("bv", [L, 1, DH], F32R)
    bo = ein("bo", [L, P, DO])
    b1, b2 = ein("b1", [L, P, FO]), ein("b2", [L, P, DO])
    ln1_g, ln1_b = ein("ln1_g", [L, P, DO]), ein("ln1_b", [L, P, DO])
    ln2_g, ln2_b = ein("ln2_g", [L, P, DO]), ein("ln2_b", [L, P, DO])
    W_out = ein("W_out", [P, DO, KD], F32R)
    b_out = ein("b_out", [P, DO])
    out = nc.dram_tensor("out", [P, B, KD], F32, kind="ExternalOutput")

    rg = [list(range(NCORES))]

    with tile.TileContext(nc) as tc:
        with tc.tile_pool(name="sb", bufs=1) as sb, \
             tc.tile_pool(name="ps", bufs=1, space="PSUM") as ps, \
             tc.tile_pool(name="dram", bufs=1, space="DRAM") as dram:

            lnp = lambda nm: sb.tile([P, FO], F32, tag="lnp", bufs=24, name=nm)

            ones_f = sb.tile([P, P], F32, tag="ones_f")
            nc.vector.memset(ones_f[:], 1.0)
            ones128_r = sb.tile([P, P], F32R, tag="ones_r")
            nc.vector.tensor_copy(ones128_r[:], ones_f[:])
            ones128_b = sb.tile([P, P], BF16, tag="ones_b")
            nc.vector.tensor_copy(ones128_b[:], ones_f[:])
            ones1_r = sb.tile([1, P], F32R, tag="ones1_r")
            nc.vector.tensor_copy(ones1_r[:], ones_f[0:1, :])
            eps_t = sb.tile([P, 1], F32, tag="eps_t")
            nc.vector.memset(eps_t[:], EPS)

            ident_f = sb.tile([P, P], F32, tag="identf")
            nc.gpsimd.affine_select(out=ident_f[:], in_=ones_f[:],
                                    compare_op=OP.is_equal, fill=0.0,
                                    base=0, channel_multiplier=1, pattern=[[-1, P]])
            ident_rep = sb.tile([P, B * P], F32R, tag="zerom")
            for bb in range(B):
                nc.vector.tensor_copy(ident_rep[:, bb * P:(bb + 1) * P], ident_f[:])

            zero_m = sb.tile([P, 512], F32, tag="zerom")
            nc.gpsimd.memset(zero_m[:], 0.0)
            mask_f = sb.tile([P, 4, 512], BF16, tag="mask")
            for m in range(4):
                nc.gpsimd.affine_select(out=mask_f[:, m, :], in_=zero_m[:],
                                        compare_op=OP.is_ge, fill=NEG,
                                        base=-128 * m, channel_multiplier=-1,
                                        pattern=[[1, 512]])

            iota_i = sb.tile([P, VC], mybir.dt.int32, tag="iota_i")
            nc.gpsimd.iota(iota_i[:], pattern=[[P, VC]], base=0, channel_multiplier=1)
            iota_f = sb.tile([P, VC], F32, tag="iota_f")
            nc.vector.tensor_copy(iota_f[:], iota_i[:])
            xrow = sb.tile([1, TL], F32, tag="ln_row", bufs=7)
            nc.sync.dma_start(xrow[:], x_f[:])
            xrow_r = sb.tile([1, TL], F32R, tag="ln_row", bufs=7)
            nc.vector.tensor_copy(xrow_r[:], xrow[:])
            xb_ps = ps.tile([P, TL], F32, tag="mm", bufs=3, name="xb_ps")
            nc.tensor.matmul(xb_ps[:], ones1_r[:], xrow_r[:], start=True, stop=True)
            onehot = sb.tile([P, VC, TL], F32R, tag="qk", bufs=2, name="onehot")
            for c in range(VC):
                nc.vector.tensor_scalar(onehot[:, c, :], xb_ps[:], iota_f[:, c:c + 1],
                                        None, OP.is_equal)

            temb = sb.tile([P, VC, DM], F32R, tag="qk", bufs=2, name="temb")
            nc.sync.dma_start(temb[:], tok_emb[:])
            pemb = sb.tile([P, 1, DM], F32R, tag="w1m", bufs=4, name="pemb")
            nc.sync.dma_start(pemb[:], pos_emb[:])
            lnig, lnib = lnp("lnig"), lnp("lnib")
            nc.sync.dma_start(lnig[:, :DO], ln_in_g[:])
            nc.sync.dma_start(lnib[:, :DO], ln_in_b[:])
            h0 = sb.tile([P, DO, TL], F32R, tag="ln_t", bufs=2, name="h0")
            for m in range(DO):
                pe = ps.tile([P, TL], F32, tag="mm", bufs=3, name="pe")
                for c in range(VC):
                    nc.tensor.matmul(pe[:], temb[:, c, m * P:(m + 1) * P],
                                     onehot[:, c, :], start=(c == 0), stop=False)
                nc.tensor.matmul(pe[:], pemb[:, 0, m * P:(m + 1) * P], ident_rep[:],
                                 start=False, stop=True)
                nc.scalar.copy(h0[:, m, :], pe[:])

            h_half = [None, None]
            ag_out = [None, None]

            def emit_ag(hh):
                ag_in = dram.tile([P, DO, TH], F32R, tag="ag_in", bufs=4,
                                  name=f"ag_in{hh}")
                ago = dram.tile([NCORES, P, DO, TH], F32R, tag="ag_out", bufs=4,
                                addr_space="Shared", name=f"ag_out{hh}")
                nc.sync.dma_start(ag_in[:], h_half[hh][:])
                nc.gpsimd.collective_compute(
                    "AllGather", OP.bypass, replica_groups=rg,
                    ins=[ag_in.opt()], outs=[ago.opt()])
                ag_out[hh] = ago

            for hh in range(2):
                h_half[hh] = _ln(nc, sb, ps,
                                 h0[:, :, hh * TH:(hh + 1) * TH],
                                 lnig[:, :DO], lnib[:, :DO], ones128_r, eps_t,
                                 f"ln_in{hh}")
                emit_ag(hh)

            for l in range(L):
                wmat = lambda nm: sb.tile([P, DO, 512], F32R, tag="w1m", bufs=4, name=nm)
                wq_t, wk_t, wv_t, wo_t = wmat("wq"), wmat("wk"), wmat("wv"), wmat("wo")
                nc.sync.dma_start(wq_t[:], Wq[l])
                nc.sync.dma_start(wk_t[:], Wk[l])
                nc.sync.dma_start(wv_t[:], Wv[l])
                nc.sync.dma_start(wo_t[:], Wo[l])
                bq_t, bk_t, bo_t = lnp("bq_t"), lnp("bk_t"), lnp("bo_t")
                nc.sync.dma_start(bq_t[:, :DO], bq[l])
                nc.sync.dma_start(bk_t[:, :DO], bk[l])
                nc.sync.dma_start(bo_t[:, :DO], bo[l])
                bv_t = sb.tile([1, DH], F32R, tag="bv_t", bufs=2, name="bv_t")
                nc.sync.dma_start(bv_t[:], bv[l])
                pbv = ps.tile([P, DH], F32, tag="mm", bufs=3, name="pbv")
                nc.tensor.matmul(pbv[:], ones1_r[:], bv_t[:], start=True, stop=True)
                bvb = sb.tile([P, DH], F32, tag="bvb", bufs=1, name="bvb")
                nc.scalar.copy(bvb[:], pbv[:])
                l1g, l1b = lnp("l1g"), lnp("l1b")
                nc.sync.dma_start(l1g[:, :DO], ln1_g[l])
                nc.sync.dma_start(l1b[:, :DO], ln1_b[l])
                b1_t, b2_t = lnp("b1_t"), lnp("b2_t")
                nc.sync.dma_start(b1_t[:], b1[l])
                nc.sync.dma_start(b2_t[:, :DO], b2[l])
                l2g, l2b = lnp("l2g"), lnp("l2b")
                nc.sync.dma_start(l2g[:, :DO], ln2_g[l])
                nc.sync.dma_start(l2b[:, :DO], ln2_b[l])

                rs_in = [dram.tile([NCORES, P, DO, 2, P], F32, tag="rs_in", bufs=4,
                                   name=f"rs_in{hh}") for hh in range(2)]
                rs_out = [dram.tile([P, DO, 2, P], F32, tag="rs_out", bufs=4,
                                    name=f"rs_out{hh}") for hh in range(2)]

                def local_half(hh, l=l, rs_out=rs_out, bo_t=bo_t, l1g=l1g, l1b=l1b,
                               b1_t=b1_t, b2_t=b2_t, l2g=l2g, l2b=l2b):
                    a_loc = sb.tile([P, DO, TH], F32, tag="aT", bufs=2, name="a_loc")
                    nc.sync.dma_start(a_loc[:],
                                      rs_out[hh].rearrange("p o b s -> p o (b s)"))
                    x1 = sb.tile([P, DO, TH], F32R, tag="aT", bufs=2, name="x1")
                    for m in range(DO):
                        nc.vector.tensor_scalar(x1[:, m, :], a_loc[:, m, :],
                                                bo_t[:, m:m + 1], None, OP.add)
                    nc.vector.tensor_tensor(x1[:], x1[:], h_half[hh][:], OP.add)
                    h1 = _ln(nc, sb, ps, x1, l1g[:, :DO], l1b[:, :DO], ones128_r,
                             eps_t, f"ln1_{l}_{hh}")
                    pz = [ps.tile([P, TH], F32, tag="acc", bufs=4, name=f"pz{m}")
                          for m in range(DO)]
                    for qq in range(4):
                        w1q = sb.tile([P, DO, 512], F32R, tag="w1m", bufs=4, name="w1q")
                        nc.sync.dma_start(w1q[:],
                                          W1[l, :, :, qq * 512:(qq + 1) * 512])
                        w2q = sb.tile([P, DO, 512], F32R, tag="w1m", bufs=4, name="w2q")
                        nc.sync.dma_start(w2q[:], W2[l, :, qq * DO:(qq + 1) * DO, :])
                        z1q = sb.tile([P, DO, TH], F32R, tag="z1q", bufs=2, name="z1q")
                        for fb in range(DO):
                            f = qq * DO + fb
                            pf = ps.tile([P, TH], F32, tag="mm", bufs=3, name="pf")
                            for c in range(DO):
                                nc.tensor.matmul(pf[:], w1q[:, c, fb * P:(fb + 1) * P],
                                                 h1[:, c, :],
                                                 start=(c == 0), stop=(c == DO - 1))
                            nc.scalar.activation(z1q[:, fb, :], pf[:], AF.Relu,
                                                 bias=b1_t[:, f:f + 1], scale=1.0)
                        for m in range(DO):
                            for cc in range(DO):
                                nc.tensor.matmul(pz[m][:],
                                                 w2q[:, cc, m * P:(m + 1) * P],
                                                 z1q[:, cc, :],
                                                 start=(qq == 0 and cc == 0),
                                                 stop=(qq == 3 and cc == DO - 1))
                    z2 = sb.tile([P, DO, TH], F32, tag="aT", bufs=2, name="z2")
                    for m in range(DO):
                        nc.scalar.activation(z2[:, m, :], pz[m][:], AF.Identity,
                                             bias=b2_t[:, m:m + 1], scale=1.0)
                    x2 = sb.tile([P, DO, TH], F32R, tag="aT", bufs=2, name="x2")
                    nc.vector.tensor_tensor(x2[:], z2[:], h1[:], OP.add)
                    h_half[hh] = _ln(nc, sb, ps, x2, l2g[:, :DO], l2b[:, :DO],
                                     ones128_r, eps_t, f"ln2_{l}_{hh}")

                for b in range(B):
                    hh = b // 2
                    hbg = []
                    for g in range(NG):
                        t = sb.tile([P, DO, 512], F32R, tag="hbg", bufs=3,
                                    name=f"hb{g}")
                        for rr in range(4):
                            r = 4 * g + rr
                            nc.sync.dma_start(
                                t[:, :, rr * P:(rr + 1) * P],
                                ag_out[hh][r, :, :, (b % 2) * P:(b % 2 + 1) * P])
                        hbg.append(t)
                    qT = sb.tile([P, DO, S], F32R, tag="qk", bufs=2, name="qT")
                    kT = sb.tile([P, DO, S], F32R, tag="qk", bufs=2, name="kT")
                    vN = sb.tile([P, S // P, DH], BF16, tag="v", bufs=1, name="vN")
                    for m in range(DO):
                        for g in range(NG):
                            sl = slice(g * 512, (g + 1) * 512)
                            pq = ps.tile([P, 512], F32, tag="mm", bufs=3, name="pq")
                            for c in range(DO):
                                nc.tensor.matmul(pq[:], wq_t[:, c, m * P:(m + 1) * P],
                                                 hbg[g][:, c, :],
                                                 start=(c == 0), stop=(c == DO - 1))
                            nc.vector.tensor_scalar(qT[:, m, sl], pq[:],
                                                    1.0 / math.sqrt(DH),
                                                    bq_t[:, m:m + 1],
                                                    OP.mult, OP.add)
                            pk = ps.tile([P, 512], F32, tag="mm", bufs=3, name="pk")
                            for c in range(DO):
                                nc.tensor.matmul(pk[:], wk_t[:, c, m * P:(m + 1) * P],
                                                 hbg[g][:, c, :],
                                                 start=(c == 0), stop=(c == DO - 1))
                            nc.vector.tensor_scalar(kT[:, m, sl], pk[:], 1.0,
                                                    bk_t[:, m:m + 1],
                                                    OP.mult, OP.add)
                    for tb in range(S // P):
                        pv = ps.tile([P, DH], F32, tag="mm", bufs=3, name="pv")
                        for c in range(DO):
                            nc.tensor.matmul(
                                pv[:],
                                hbg[tb // 4][:, c, (tb % 4) * P:(tb % 4 + 1) * P],
                                wv_t[:, c, :], start=(c == 0), stop=(c == DO - 1))
                        nc.vector.tensor_tensor(vN[:, tb, :], pv[:], bvb[:], OP.add)

                    oT = sb.tile([P, DO, S], F32R, tag="oT", bufs=1, name="oT")
                    for g in range(NG):
                        nj = 4 * g + 4
                        sl = slice(g * 512, (g + 1) * 512)
                        attnT = sb.tile([P, S // P, 512], BF16, tag="attnT", bufs=1,
                                        name="attnT")
                        pden = ps.tile([P, 512], F32, tag="ln_ps", bufs=1, name="pden")
                        lo = lambda j: max(0, (j - 4 * g) * P)
                        for j in range(nj):
                            o = lo(j)
                            sc = ps.tile([P, 512], F32, tag="mm", bufs=3, name="sc")
                            for c in range(DO):
                                nc.tensor.matmul(sc[:, o:], kT[:, c, j * P:(j + 1) * P],
                                                 qT[:, c, g * 512 + o:(g + 1) * 512],
                                                 start=(c == 0), stop=(c == DO - 1))
                            if j >= 4 * g:
                                m = j - 4 * g
                                nc.vector.tensor_tensor(sc[:, o:], sc[:, o:],
                                                        mask_f[:, m, o:], OP.add)
                            nc.scalar.activation(attnT[:, j, o:], sc[:, o:], AF.Exp,
                                                 bias=0.0, scale=1.0)
                            nc.tensor.matmul(pden[:, o:], ones128_b[:],
                                             attnT[:, j, o:],
                                             start=(j == 0), stop=(j == nj - 1))
                        recip = sb.tile([P, 512], F32, tag="ln_row", bufs=7,
                                        name="recip")
                        nc.vector.reciprocal(recip[:], pden[:])
                        for d in range(DO):
                            po = ps.tile([P, 512], F32, tag="acc", bufs=4, name="po")
                            for j in range(nj):
                                o = lo(j)
                                nc.tensor.matmul(po[:, o:], vN[:, j, d * P:(d + 1) * P],
                                                 attnT[:, j, o:],
                                                 start=(j == 0), stop=(j == nj - 1))
                            nc.vector.tensor_tensor(oT[:, d, sl], po[:], recip[:],
                                                    OP.mult)

                    for g in range(NG):
                        sl = slice(g * 512, (g + 1) * 512)
                        aT = sb.tile([P, DO, 512], F32, tag="aT", bufs=2, name="aT")
                        for m in range(DO):
                            pa = ps.tile([P, 512], F32, tag="mm", bufs=3, name="pa")
                            for c in range(DO):
                                nc.tensor.matmul(pa[:], wo_t[:, c, m * P:(m + 1) * P],
                                                 oT[:, c, sl],
                                                 start=(c == 0), stop=(c == DO - 1))
                            nc.scalar.copy(aT[:, m, :], pa[:])
                        for q in range(4):
                            nc.sync.dma_start(rs_in[hh][4 * g + q, :, :, b % 2, :],
                                              aT[:, :, q * P:(q + 1) * P])

                    if b % 2 == 1:
                        nc.gpsimd.collective_compute(
                            "ReduceScatter", OP.add, replica_groups=rg,
                            ins=[rs_in[hh].opt()], outs=[rs_out[hh].opt()])
                for hh in range(2):
                    local_half(hh)
                    if l < L - 1:
                        emit_ag(hh)

            wout = sb.tile([P, DO, KD], F32R, tag="w1m", bufs=4, name="wout")
            nc.sync.dma_start(wout[:], W_out[:])
            bout = lnp("bout")
            nc.sync.dma_start(bout[:, :DO], b_out[:])
            ident_t = sb.tile([P, P], F32, tag="identf")
            make_identity(nc, ident_t)
            out_sb = sb.tile([P, B, KD], F32, tag="oT", bufs=1, name="out_sb")
            for hh in range(2):
                zT = sb.tile([P, DO, TH], F32, tag="aT", bufs=2, name="zT")
                for m in range(DO):
                    pu = ps.tile([P, TH], F32, tag="mm", bufs=3, name="pu")
                    for c in range(DO):
                        nc.tensor.matmul(pu[:], wout[:, c, m * P:(m + 1) * P],
                                         h_half[hh][:, c, :],
                                         start=(c == 0), stop=(c == DO - 1))
                    nc.scalar.activation(zT[:, m, :], pu[:], AF.Identity,
                                         bias=bout[:, m:m + 1], scale=1.0)
                for kb in range(DO):
                    for tb in range(2):
                        pt = ps.tile([P, P], F32, tag="mm", bufs=3, name="pt")
                        nc.tensor.transpose(pt[:], zT[:, kb, tb * P:(tb + 1) * P],
                                            ident_t[:])
                        nc.scalar.copy(out_sb[:, 2 * hh + tb, kb * P:(kb + 1) * P],
                                       pt[:])
            nc.sync.dma_start(out[:], out_sb[:])

    nc.compile()
    return nc


_NC_CACHE = None


def _get_nc():
    global _NC_CACHE
    if _NC_CACHE is None:
        _NC_CACHE = build_nc()
    return _NC_CACHE


def _lhsT(w):
    Kd, Nd = w.shape
    return np.ascontiguousarray(w.reshape(Kd // P, P, Nd).transpose(1, 0, 2))


def _ppart(v):
    return np.ascontiguousarray(v.reshape(-1, P).T)


def make_in_maps(inputs):
    inp = {k: np.asarray(v) for k, v in inputs.items()}
    x = inp["x"].astype(np.int64)
    f32 = np.float32

    tok_pad = np.zeros((VP, DM), f32)
    tok_pad[:VOCAB] = inp["tok_emb"].astype(f32)
    tok_l = _lhsT(tok_pad)
    stack = lambda fn: np.stack([fn(l) for l in range(L)])

    in_maps = []
    for r in range(NCORES):
        h0c, h1c = r * DH, (r + 1) * DH
        m = {
            "x_f": np.ascontiguousarray(
                x[:, r * P:(r + 1) * P].reshape(-1).astype(f32)[None, :]),
            "tok_emb": tok_l,
            "pos_emb": np.ascontiguousarray(
                inp["pos_emb"][r * P:(r + 1) * P].astype(f32))[:, None, :],
            "ln_in_g": _ppart(inp["ln_in_g"].astype(f32)),
            "ln_in_b": _ppart(inp["ln_in_b"].astype(f32)),
            "Wq": stack(lambda l: _lhsT(inp["Wq"][l, :, h0c:h1c].astype(f32))),
            "Wk": stack(lambda l: _lhsT(inp["Wk"][l, :, h0c:h1c].astype(f32))),
            "Wv": stack(lambda l: _lhsT(inp["Wv"][l, :, h0c:h1c].astype(f32))),
            "Wo": stack(lambda l: _lhsT(inp["Wo"][l, h0c:h1c, :].astype(f32))),
            "W1": stack(lambda l: _lhsT(inp["W1"][l].astype(f32))),
            "W2": stack(lambda l: _lhsT(inp["W2"][l].astype(f32))),
            "bq": stack(lambda l: _ppart(
                inp["bq"][l, h0c:h1c].astype(f32) / np.float32(math.sqrt(DH)))),
            "bk": stack(lambda l: _ppart(inp["bk"][l, h0c:h1c].astype(f32))),
            "bv": np.ascontiguousarray(inp["bv"][:, h0c:h1c].astype(f32)[:, None, :]),
            "bo": stack(lambda l: _ppart(inp["bo"][l].astype(f32))),
            "b1": stack(lambda l: _ppart(inp["b1"][l].astype(f32))),
            "b2": stack(lambda l: _ppart(inp["b2"][l].astype(f32))),
            "ln1_g": stack(lambda l: _ppart(inp["ln1_g"][l].astype(f32))),
            "ln1_b": stack(lambda l: _ppart(inp["ln1_b"][l].astype(f32))),
            "ln2_g": stack(lambda l: _ppart(inp["ln2_g"][l].astype(f32))),
            "ln2_b": stack(lambda l: _ppart(inp["ln2_b"][l].astype(f32))),
            "W_out": _lhsT(inp["W_out"].astype(f32)),
            "b_out": _ppart(inp["b_out"].astype(f32)),
        }
        in_maps.append(m)
    return in_maps


def assemble_output(results):
    full = np.zeros((B, S, KD), np.float32)
    for r in range(NCORES):
        o = results[r]["out"]
        for b in range(B):
            full[b, r * P:(r + 1) * P, :] = o[:, b, :]
    return full


def kernel(**inputs):
    in_maps = make_in_maps(inputs)
    nc = _get_nc()
    res = run_bass_kernel_spmd(nc, in_maps, core_ids=list(range(NCORES)))
    return assemble_output(res.results)


# revision 12
# speedup vs baseline: 1.1655x; 1.1655x over previous
import math
import sys

import numpy as np

sys.path.insert(0, "/opt/trn_rl_repo")
import ml_dtypes

BF16NP = ml_dtypes.bfloat16

import concourse.bass as bass
import concourse.mybir as mybir
import concourse.tile as tile
from concourse import bacc
from concourse.bass_utils import run_bass_kernel_spmd
from concourse.masks import make_identity

P = 128
B, S, DM, NH, L, DFF, KD, VOCAB = 4, 1024, 512, 8, 6, 2048, 512, 522
DH = DM
DO = DM // P
VP = 640
VC = VP // P
TL = B * P
TH = TL // 2
FO = DFF // P
NG = S // 512
NCORES = 8
EPS = 1e-5
F32 = mybir.dt.float32
F32R = mybir.dt.float32r
BF16 = mybir.dt.bfloat16
NEG = -30000.0
AF = mybir.ActivationFunctionType
OP = mybir.AluOpType


def _ln(nc, sb, ps, xT, g_ap, b_ap, ones128_r, eps_t, name):
    T = xT.shape[2]
    sq = sb.tile([P, DO, T], F32R, tag="ln_t", bufs=2, name=f"{name}_sq")
    nc.vector.tensor_tensor(sq[:], xT[:], xT[:], OP.mult)
    s1 = ps.tile([P, T], F32, tag="ln_ps", bufs=1, name=f"{name}_s1")
    s2 = ps.tile([P, T], F32, tag="ln_ps", bufs=1, name=f"{name}_s2")
    for c in range(DO):
        nc.tensor.matmul(s1[:], ones128_r[:], xT[:, c, :], start=(c == 0), stop=(c == DO - 1))
    for c in range(DO):
        nc.tensor.matmul(s2[:], ones128_r[:], sq[:, c, :], start=(c == 0), stop=(c == DO - 1))
    row = lambda nm: sb.tile([P, T], F32, tag="ln_row", bufs=7, name=f"{name}_{nm}")
    mean = row("mean")
    nc.vector.tensor_scalar(mean[:], s1[:], 1.0 / DM, None, OP.mult)
    m2 = row("m2")
    nc.vector.tensor_tensor(m2[:], mean[:], mean[:], OP.mult)
    ex2 = row("ex2")
    nc.vector.tensor_scalar(ex2[:], s2[:], 1.0 / DM, None, OP.mult)
    var = row("var")
    nc.vector.tensor_tensor(var[:], ex2[:], m2[:], OP.subtract)
    std = row("std")
    nc.scalar.activation(std[:], var[:], AF.Sqrt, bias=eps_t[:], scale=1.0)
    rstd = row("rstd")
    nc.vector.reciprocal(rstd[:], std[:])
    mrs = row("mrs")
    nc.vector.tensor_tensor(mrs[:], mean[:], rstd[:], OP.mult)
    out = sb.tile([P, DO, T], F32R, tag="ln_out", bufs=4, name=f"{name}_out")
    u = sb.tile([P, DO, T], F32, tag="ln_t", bufs=2, name=f"{name}_u")
    for c in range(DO):
        nc.vector.tensor_tensor(u[:, c, :], xT[:, c, :], rstd[:], OP.mult)
        nc.vector.tensor_tensor(u[:, c, :], u[:, c, :], mrs[:], OP.subtract)
        nc.vector.tensor_scalar(out[:, c, :], u[:, c, :], g_ap[:, c:c + 1],
                                b_ap[:, c:c + 1], OP.mult, OP.add)
    return out


def build_nc():
    nc = bacc.Bacc(None, target_bir_lowering=False, debug=False)

    ein = lambda nm, shp, dt=F32: nc.dram_tensor(nm, shp, dt, kind="ExternalInput")
    x_f = ein("x_f", [1, TL])
    tok_emb = ein("tok_emb", [P, VC, DM], F32R)
    pos_emb = ein("pos_emb", [P, 1, DM], F32R)
    ln_in_g, ln_in_b = ein("ln_in_g", [P, DO]), ein("ln_in_b", [P, DO])
    Wq, Wk, Wv = (ein(n, [L, P, DO, DH], BF16) for n in ("Wq", "Wk", "Wv"))
    Wo = ein("Wo", [L, P, DO, DM], BF16)
    W1 = ein("W1", [L, P, DO, DFF], BF16)
    W2 = ein("W2", [L, P, FO, DM], BF16)
    bq, bk = ein("bq", [L, P, DO]), ein("bk", [L, P, DO])
    bv = ein("bv", [L, 1, DH], F32R)
    bo = ein("bo", [L, P, DO])
    b1, b2 = ein("b1", [L, P, FO]), ein("b2", [L, P, DO])
    ln1_g, ln1_b = ein("ln1_g", [L, P, DO]), ein("ln1_b", [L, P, DO])
    ln2_g, ln2_b = ein("ln2_g", [L, P, DO]), ein("ln2_b", [L, P, DO])
    W_out = ein("W_out", [P, DO, KD], F32R)
    b_out = ein("b_out", [P, DO])
    out = nc.dram_tensor("out", [P, B, KD], F32, kind="ExternalOutput")

    rg = [list(range(NCORES))]

    with tile.TileContext(nc) as tc:
        with tc.tile_pool(name="sb", bufs=1) as sb, \
             tc.tile_pool(name="ps", bufs=1, space="PSUM") as ps, \
             tc.tile_pool(name="dram", bufs=1, space="DRAM") as dram:

            lnp = lambda nm: sb.tile([P, FO], F32, tag="lnp", bufs=24, name=nm)

            ones_f = sb.tile([P, P], F32, tag="ones_f")
            nc.vector.memset(ones_f[:], 1.0)
            ones128_r = sb.tile([P, P], F32R, tag="ones_r")
            nc.vector.tensor_copy(ones128_r[:], ones_f[:])
            ones128_b = sb.tile([P, P], BF16, tag="ones_b")
            nc.vector.tensor_copy(ones128_b[:], ones_f[:])
            ones1_r = sb.tile([1, P], F32R, tag="ones1_r")
            nc.vector.tensor_copy(ones1_r[:], ones_f[0:1, :])
            eps_t = sb.tile([P, 1], F32, tag="eps_t")
            nc.vector.memset(eps_t[:], EPS)

            ident_f = sb.tile([P, P], F32, tag="identf")
            nc.gpsimd.affine_select(out=ident_f[:], in_=ones_f[:],
                                    compare_op=OP.is_equal, fill=0.0,
                                    base=0, channel_multiplier=1, pattern=[[-1, P]])
            ident_rep = sb.tile([P, B * P], F32R, tag="zerom")
            for bb in range(B):
                nc.vector.tensor_copy(ident_rep[:, bb * P:(bb + 1) * P], ident_f[:])

            zero_m = sb.tile([P, 512], F32, tag="zerom")
            nc.gpsimd.memset(zero_m[:], 0.0)
            mask_f = sb.tile([P, 4, 512], BF16, tag="mask")
            for m in range(4):
                nc.gpsimd.affine_select(out=mask_f[:, m, :], in_=zero_m[:],
                                        compare_op=OP.is_ge, fill=NEG,
                                        base=-128 * m, channel_multiplier=-1,
                                        pattern=[[1, 512]])

            iota_i = sb.tile([P, VC], mybir.dt.int32, tag="iota_i")
            nc.gpsimd.iota(iota_i[:], pattern=[[P, VC]], base=0, channel_multiplier=1)
            iota_f = sb.tile([P, VC], F32, tag="iota_f")
            nc.vector.tensor_copy(iota_f[:], iota_i[:])
            xrow = sb.tile([1, TL], F32, tag="ln_row", bufs=7)
            nc.sync.dma_start(xrow[:], x_f[:])
            xrow_r = sb.tile([1, TL], F32R, tag="ln_row", bufs=7)
            nc.vector.tensor_copy(xrow_r[:], xrow[:])
            xb_ps = ps.tile([P, TL], F32, tag="mm", bufs=3, name="xb_ps")
            nc.tensor.matmul(xb_ps[:], ones1_r[:], xrow_r[:], start=True, stop=True)
            onehot = sb.tile([P, VC, TL], F32R, tag="qk", bufs=2, name="onehot")
            for c in range(VC):
                nc.vector.tensor_scalar(onehot[:, c, :], xb_ps[:], iota_f[:, c:c + 1],
                                        None, OP.is_equal)

            temb = sb.tile([P, VC, DM], F32R, tag="qk", bufs=2, name="temb")
            nc.sync.dma_start(temb[:], tok_emb[:])
            pemb = sb.tile([P, 1, DM], F32R, tag="bvb", bufs=2, name="pemb")
            nc.sync.dma_start(pemb[:], pos_emb[:])
            lnig, lnib = lnp("lnig"), lnp("lnib")
            nc.sync.dma_start(lnig[:, :DO], ln_in_g[:])
            nc.sync.dma_start(lnib[:, :DO], ln_in_b[:])
            h0 = sb.tile([P, DO, TL], F32R, tag="ln_t", bufs=2, name="h0")
            for m in range(DO):
                pe = ps.tile([P, TL], F32, tag="mm", bufs=3, name="pe")
                for c in range(VC):
                    nc.tensor.matmul(pe[:], temb[:, c, m * P:(m + 1) * P],
                                     onehot[:, c, :], start=(c == 0), stop=False)
                nc.tensor.matmul(pe[:], pemb[:, 0, m * P:(m + 1) * P], ident_rep[:],
                                 start=False, stop=True)
                nc.scalar.copy(h0[:, m, :], pe[:])

            h_half = [None, None]
            ag_out = [None, None]

            def emit_ag(hh):
                ag_in = dram.tile([P, DO, TH], BF16, tag="ag_in", bufs=4,
                                  name=f"ag_in{hh}")
                ago = dram.tile([NCORES, P, DO, TH], BF16, tag="ag_out", bufs=4,
                                addr_space="Shared", name=f"ag_out{hh}")
                h16 = sb.tile([P, DO, TH], BF16, tag="h16", bufs=2, name="h16")
                nc.vector.tensor_copy(h16[:], h_half[hh][:])
                nc.sync.dma_start(ag_in[:], h16[:])
                nc.gpsimd.collective_compute(
                    "AllGather", OP.bypass, replica_groups=rg,
                    ins=[ag_in.opt()], outs=[ago.opt()])
                ag_out[hh] = ago

            for hh in range(2):
                h_half[hh] = _ln(nc, sb, ps,
                                 h0[:, :, hh * TH:(hh + 1) * TH],
                                 lnig[:, :DO], lnib[:, :DO], ones128_r, eps_t,
                                 f"ln_in{hh}")
                emit_ag(hh)

            for l in range(L):
                wmat = lambda nm: sb.tile([P, DO, 512], BF16, tag="w1m", bufs=6, name=nm)
                wq_t, wk_t, wv_t, wo_t = wmat("wq"), wmat("wk"), wmat("wv"), wmat("wo")
                nc.sync.dma_start(wq_t[:], Wq[l])
                nc.sync.dma_start(wk_t[:], Wk[l])
                nc.sync.dma_start(wv_t[:], Wv[l])
                nc.sync.dma_start(wo_t[:], Wo[l])
                bq_t, bk_t, bo_t = lnp("bq_t"), lnp("bk_t"), lnp("bo_t")
                nc.sync.dma_start(bq_t[:, :DO], bq[l])
                nc.sync.dma_start(bk_t[:, :DO], bk[l])
                nc.sync.dma_start(bo_t[:, :DO], bo[l])
                bv_t = sb.tile([1, DH], F32R, tag="bv_t", bufs=2, name="bv_t")
                nc.sync.dma_start(bv_t[:], bv[l])
                pbv = ps.tile([P, DH], F32, tag="mm", bufs=3, name="pbv")
                nc.tensor.matmul(pbv[:], ones1_r[:], bv_t[:], start=True, stop=True)
                bvb = sb.tile([P, DH], F32, tag="bvb", bufs=2, name="bvb")
                nc.scalar.copy(bvb[:], pbv[:])
                l1g, l1b = lnp("l1g"), lnp("l1b")
                nc.sync.dma_start(l1g[:, :DO], ln1_g[l])
                nc.sync.dma_start(l1b[:, :DO], ln1_b[l])
                b1_t, b2_t = lnp("b1_t"), lnp("b2_t")
                nc.sync.dma_start(b1_t[:], b1[l])
                nc.sync.dma_start(b2_t[:, :DO], b2[l])
                l2g, l2b = lnp("l2g"), lnp("l2b")
                nc.sync.dma_start(l2g[:, :DO], ln2_g[l])
                nc.sync.dma_start(l2b[:, :DO], ln2_b[l])

                rs_in = [dram.tile([NCORES, P, DO, 2, P], BF16, tag="rs_in", bufs=4,
                                   name=f"rs_in{hh}") for hh in range(2)]
                rs_out = [dram.tile([P, DO, 2, P], BF16, tag="rs_out", bufs=4,
                                    name=f"rs_out{hh}") for hh in range(2)]

                def local_half(hh, l=l, rs_out=rs_out, bo_t=bo_t, l1g=l1g, l1b=l1b,
                               b1_t=b1_t, b2_t=b2_t, l2g=l2g, l2b=l2b):
                    a_loc = sb.tile([P, DO, TH], BF16, tag="h16", bufs=2, name="a_loc")
                    nc.sync.dma_start(a_loc[:],
                                      rs_out[hh].rearrange("p o b s -> p o (b s)"))
                    x1 = sb.tile([P, DO, TH], F32R, tag="aT", bufs=2, name="x1")
                    for m in range(DO):
                        nc.vector.tensor_scalar(x1[:, m, :], a_loc[:, m, :],
                                                bo_t[:, m:m + 1], None, OP.add)
                    nc.vector.tensor_tensor(x1[:], x1[:], h_half[hh][:], OP.add)
                    h1 = _ln(nc, sb, ps, x1, l1g[:, :DO], l1b[:, :DO], ones128_r,
                             eps_t, f"ln1_{l}_{hh}")
                    h1b = sb.tile([P, DO, TH], BF16, tag="h16", bufs=2, name="h1b")
                    nc.vector.tensor_copy(h1b[:], h1[:])
                    pz = [ps.tile([P, TH], F32, tag="acc", bufs=4, name=f"pz{m}")
                          for m in range(DO)]
                    for qq in range(4):
                        w1q = sb.tile([P, DO, 512], BF16, tag="w1m", bufs=6, name="w1q")
                        nc.sync.dma_start(w1q[:],
                                          W1[l, :, :, qq * 512:(qq + 1) * 512])
                        w2q = sb.tile([P, DO, 512], BF16, tag="w1m", bufs=6, name="w2q")
                        nc.sync.dma_start(w2q[:], W2[l, :, qq * DO:(qq + 1) * DO, :])
                        z1q = sb.tile([P, DO, TH], BF16, tag="z1q", bufs=2, name="z1q")
                        for fb in range(DO):
                            f = qq * DO + fb
                            pf = ps.tile([P, TH], F32, tag="mm", bufs=3, name="pf")
                            for c in range(DO):
                                nc.tensor.matmul(pf[:], w1q[:, c, fb * P:(fb + 1) * P],
                                                 h1b[:, c, :],
                                                 start=(c == 0), stop=(c == DO - 1))
                            nc.scalar.activation(z1q[:, fb, :], pf[:], AF.Relu,
                                                 bias=b1_t[:, f:f + 1], scale=1.0)
                        for m in range(DO):
                            for cc in range(DO):
                                nc.tensor.matmul(pz[m][:],
                                                 w2q[:, cc, m * P:(m + 1) * P],
                                                 z1q[:, cc, :],
                                                 start=(qq == 0 and cc == 0),
                                                 stop=(qq == 3 and cc == DO - 1))
                    z2 = sb.tile([P, DO, TH], F32, tag="aT", bufs=2, name="z2")
                    for m in range(DO):
                        nc.scalar.activation(z2[:, m, :], pz[m][:], AF.Identity,
                                             bias=b2_t[:, m:m + 1], scale=1.0)
                    x2 = sb.tile([P, DO, TH], F32R, tag="aT", bufs=2, name="x2")
                    nc.vector.tensor_tensor(x2[:], z2[:], h1[:], OP.add)
                    h_half[hh] = _ln(nc, sb, ps, x2, l2g[:, :DO], l2b[:, :DO],
                                     ones128_r, eps_t, f"ln2_{l}_{hh}")

                for b in range(B):
                    hh = b // 2
                    hbg = []
                    for g in range(NG):
                        t = sb.tile([P, DO, 512], BF16, tag="hbg", bufs=4,
                                    name=f"hb{g}")
                        for rr in range(4):
                            r = 4 * g + rr
                            nc.sync.dma_start(
                                t[:, :, rr * P:(rr + 1) * P],
                                ag_out[hh][r, :, :, (b % 2) * P:(b % 2 + 1) * P])
                        hbg.append(t)
                    qT = sb.tile([P, DO, S], F32R, tag="qk", bufs=2, name="qT")
                    kT = sb.tile([P, DO, S], F32R, tag="qk", bufs=2, name="kT")
                    vN = sb.tile([P, S // P, DH], BF16, tag="v", bufs=2, name="vN")
                    for m in range(DO):
                        for g in range(NG):
                            sl = slice(g * 512, (g + 1) * 512)
                            pq = ps.tile([P, 512], F32, tag="mm", bufs=3, name="pq")
                            for c in range(DO):
                                nc.tensor.matmul(pq[:], wq_t[:, c, m * P:(m + 1) * P],
                                                 hbg[g][:, c, :],
                                                 start=(c == 0), stop=(c == DO - 1))
                            nc.vector.tensor_scalar(qT[:, m, sl], pq[:],
                                                    1.0 / math.sqrt(DH),
                                                    bq_t[:, m:m + 1],
                                                    OP.mult, OP.add)
                            pk = ps.tile([P, 512], F32, tag="mm", bufs=3, name="pk")
                            for c in range(DO):
                                nc.tensor.matmul(pk[:], wk_t[:, c, m * P:(m + 1) * P],
                                                 hbg[g][:, c, :],
                                                 start=(c == 0), stop=(c == DO - 1))
                            nc.vector.tensor_scalar(kT[:, m, sl], pk[:], 1.0,
                                                    bk_t[:, m:m + 1],
                                                    OP.mult, OP.add)
                    for tb in range(S // P):
                        pv = ps.tile([P, DH], F32, tag="mm", bufs=3, name="pv")
                        for c in range(DO):
                            nc.tensor.matmul(
                                pv[:],
                                hbg[tb // 4][:, c, (tb % 4) * P:(tb % 4 + 1) * P],
                                wv_t[:, c, :], start=(c == 0), stop=(c == DO - 1))
                        nc.vector.tensor_tensor(vN[:, tb, :], pv[:], bvb[:], OP.add)

                    oT = sb.tile([P, DO, S], BF16, tag="oT", bufs=1, name="oT")
                    for g in range(NG):
                        nj = 4 * g + 4
                        sl = slice(g * 512, (g + 1) * 512)
                        attnT = sb.tile([P, S // P, 512], BF16, tag="attnT", bufs=2,
                                        name="attnT")
                        pden = ps.tile([P, 512], F32, tag="ln_ps", bufs=1, name="pden")
                        lo = lambda j: max(0, (j - 4 * g) * P)
                        for j in range(nj):
                            o = lo(j)
                            sc = ps.tile([P, 512], F32, tag="mm", bufs=3, name="sc")
                            for c in range(DO):
                                nc.tensor.matmul(sc[:, o:], kT[:, c, j * P:(j + 1) * P],
                                                 qT[:, c, g * 512 + o:(g + 1) * 512],
                                                 start=(c == 0), stop=(c == DO - 1))
                            if j >= 4 * g:
                                m = j - 4 * g
                                nc.vector.tensor_tensor(sc[:, o:], sc[:, o:],
                                                        mask_f[:, m, o:], OP.add)
                            nc.scalar.activation(attnT[:, j, o:], sc[:, o:], AF.Exp,
                                                 bias=0.0, scale=1.0)
                            nc.tensor.matmul(pden[:, o:], ones128_b[:],
                                             attnT[:, j, o:],
                                             start=(j == 0), stop=(j == nj - 1))
                        recip = sb.tile([P, 512], F32, tag="ln_row", bufs=7,
                                        name="recip")
                        nc.vector.reciprocal(recip[:], pden[:])
                        for d in range(DO):
                            po = ps.tile([P, 512], F32, tag="acc", bufs=4, name="po")
                            for j in range(nj):
                                o = lo(j)
                                nc.tensor.matmul(po[:, o:], vN[:, j, d * P:(d + 1) * P],
                                                 attnT[:, j, o:],
                                                 start=(j == 0), stop=(j == nj - 1))
                            nc.vector.tensor_tensor(oT[:, d, sl], po[:], recip[:],
                                                    OP.mult)

                    for g in range(NG):
                        sl = slice(g * 512, (g + 1) * 512)
                        aT = sb.tile([P, DO, 512], BF16, tag="a16", bufs=2, name="aT")
                        for m in range(DO):
                            pa = ps.tile([P, 512], F32, tag="mm", bufs=3, name="pa")
                            for c in range(DO):
                                nc.tensor.matmul(pa[:], wo_t[:, c, m * P:(m + 1) * P],
                                                 oT[:, c, sl],
                                                 start=(c == 0), stop=(c == DO - 1))
                            nc.scalar.copy(aT[:, m, :], pa[:])
                        for q in range(4):
                            nc.sync.dma_start(rs_in[hh][4 * g + q, :, :, b % 2, :],
                                              aT[:, :, q * P:(q + 1) * P])

                    if b % 2 == 1:
                        nc.gpsimd.collective_compute(
                            "ReduceScatter", OP.add, replica_groups=rg,
                            ins=[rs_in[hh].opt()], outs=[rs_out[hh].opt()])
                for hh in range(2):
                    local_half(hh)
                    if l < L - 1:
                        emit_ag(hh)

            wout = sb.tile([P, DO, KD], F32R, tag="aT", bufs=2, name="wout")
            nc.sync.dma_start(wout[:], W_out[:])
            bout = lnp("bout")
            nc.sync.dma_start(bout[:, :DO], b_out[:])
            ident_t = sb.tile([P, P], F32, tag="identf")
            make_identity(nc, ident_t)
            out_sb = sb.tile([P, B, KD], F32, tag="oT", bufs=1, name="out_sb")
            for hh in range(2):
                zT = sb.tile([P, DO, TH], F32, tag="aT", bufs=2, name="zT")
                for m in range(DO):
                    pu = ps.tile([P, TH], F32, tag="mm", bufs=3, name="pu")
                    for c in range(DO):
                        nc.tensor.matmul(pu[:], wout[:, c, m * P:(m + 1) * P],
                                         h_half[hh][:, c, :],
                                         start=(c == 0), stop=(c == DO - 1))
                    nc.scalar.activation(zT[:, m, :], pu[:], AF.Identity,
                                         bias=bout[:, m:m + 1], scale=1.0)
                for kb in range(DO):
                    for tb in range(2):
                        pt = ps.tile([P, P], F32, tag="mm", bufs=3, name="pt")
                        nc.tensor.transpose(pt[:], zT[:, kb, tb * P:(tb + 1) * P],
                                            ident_t[:])
                        nc.scalar.copy(out_sb[:, 2 * hh + tb, kb * P:(kb + 1) * P],
                                       pt[:])
            nc.sync.dma_start(out[:], out_sb[:])

    nc.compile()
    return nc


_NC_CACHE = None


def _get_nc():
    global _NC_CACHE
    if _NC_CACHE is None:
        _NC_CACHE = build_nc()
    return _NC_CACHE


def _lhsT(w):
    Kd, Nd = w.shape
    return np.ascontiguousarray(w.reshape(Kd // P, P, Nd).transpose(1, 0, 2))


def _ppart(v):
    return np.ascontiguousarray(v.reshape(-1, P).T)


def make_in_maps(inputs):
    inp = {k: np.asarray(v) for k, v in inputs.items()}
    x = inp["x"].astype(np.int64)
    f32 = np.float32

    tok_pad = np.zeros((VP, DM), f32)
    tok_pad[:VOCAB] = inp["tok_emb"].astype(f32)
    tok_l = _lhsT(tok_pad)
    stack = lambda fn: np.stack([fn(l) for l in range(L)])

    in_maps = []
    for r in range(NCORES):
        h0c, h1c = r * DH, (r + 1) * DH
        m = {
            "x_f": np.ascontiguousarray(
                x[:, r * P:(r + 1) * P].reshape(-1).astype(f32)[None, :]),
            "tok_emb": tok_l,
            "pos_emb": np.ascontiguousarray(
                inp["pos_emb"][r * P:(r + 1) * P].astype(f32))[:, None, :],
            "ln_in_g": _ppart(inp["ln_in_g"].astype(f32)),
            "ln_in_b": _ppart(inp["ln_in_b"].astype(f32)),
            "Wq": stack(lambda l: _lhsT(inp["Wq"][l, :, h0c:h1c].astype(f32)).astype(BF16NP)),
            "Wk": stack(lambda l: _lhsT(inp["Wk"][l, :, h0c:h1c].astype(f32)).astype(BF16NP)),
            "Wv": stack(lambda l: _lhsT(inp["Wv"][l, :, h0c:h1c].astype(f32)).astype(BF16NP)),
            "Wo": stack(lambda l: _lhsT(inp["Wo"][l, h0c:h1c, :].astype(f32)).astype(BF16NP)),
            "W1": stack(lambda l: _lhsT(inp["W1"][l].astype(f32)).astype(BF16NP)),
            "W2": stack(lambda l: _lhsT(inp["W2"][l].astype(f32)).astype(BF16NP)),
            "bq": stack(lambda l: _ppart(
                inp["bq"][l, h0c:h1c].astype(f32) / np.float32(math.sqrt(DH)))),
            "bk": stack(lambda l: _ppart(inp["bk"][l, h0c:h1c].astype(f32))),
            "bv": np.ascontiguousarray(inp["bv"][:, h0c:h1c].astype(f32)[:, None, :]),
            "bo": stack(lambda l: _ppart(inp["bo"][l].astype(f32))),
            "b1": stack(lambda l: _ppart(inp["b1"][l].astype(f32))),
            "b2": stack(lambda l: _ppart(inp["b2"][l].astype(f32))),
            "ln1_g": stack(lambda l: _ppart(inp["ln1_g"][l].astype(f32))),
            "ln1_b": stack(lambda l: _ppart(inp["ln1_b"][l].astype(f32))),
            "ln2_g": stack(lambda l: _ppart(inp["ln2_g"][l].astype(f32))),
            "ln2_b": stack(lambda l: _ppart(inp["ln2_b"][l].astype(f32))),
            "W_out": _lhsT(inp["W_out"].astype(f32)),
            "b_out": _ppart(inp["b_out"].astype(f32)),
        }
        in_maps.append(m)
    return in_maps


def assemble_output(results):
    full = np.zeros((B, S, KD), np.float32)
    for r in range(NCORES):
        o = results[r]["out"]
        for b in range(B):
            full[b, r * P:(r + 1) * P, :] = o[:, b, :]
    return full


def kernel(**inputs):
    in_maps = make_in_maps(inputs)
    nc = _get_nc()
    res = run_bass_kernel_spmd(nc, in_maps, core_ids=list(range(NCORES)))
    return assemble_output(res.results)


# revision 17
# speedup vs baseline: 1.1702x; 1.0041x over previous
import math
import sys

import numpy as np

sys.path.insert(0, "/opt/trn_rl_repo")
import ml_dtypes

BF16NP = ml_dtypes.bfloat16

import concourse.bass as bass
import concourse.mybir as mybir
import concourse.tile as tile
from concourse import bacc
from concourse.bass_utils import run_bass_kernel_spmd
from concourse.masks import make_identity

P = 128
B, S, DM, NH, L, DFF, KD, VOCAB = 4, 1024, 512, 8, 6, 2048, 512, 522
DH = DM
DO = DM // P
VP = 640
VC = VP // P
TL = B * P
TH = TL // 2
FO = DFF // P
NG = S // 512
NCORES = 8
EPS = 1e-5
F32 = mybir.dt.float32
F32R = mybir.dt.float32r
BF16 = mybir.dt.bfloat16
NEG = -30000.0
AF = mybir.ActivationFunctionType
OP = mybir.AluOpType


def _ln(nc, sb, ps, xT, g_ap, b_ap, ones128_r, eps_t, name):
    T = xT.shape[2]
    sq = sb.tile([P, DO, T], F32R, tag="ln_t", bufs=2, name=f"{name}_sq")
    nc.vector.tensor_tensor(sq[:], xT[:], xT[:], OP.mult)
    s1 = ps.tile([P, T], F32, tag="ln_ps", bufs=1, name=f"{name}_s1")
    s2 = ps.tile([P, T], F32, tag="ln_ps", bufs=1, name=f"{name}_s2")
    for c in range(DO):
        nc.tensor.matmul(s1[:], ones128_r[:], xT[:, c, :], start=(c == 0), stop=(c == DO - 1))
    for c in range(DO):
        nc.tensor.matmul(s2[:], ones128_r[:], sq[:, c, :], start=(c == 0), stop=(c == DO - 1))
    row = lambda nm: sb.tile([P, T], F32, tag="ln_row", bufs=7, name=f"{name}_{nm}")
    mean = row("mean")
    nc.vector.tensor_scalar(mean[:], s1[:], 1.0 / DM, None, OP.mult)
    m2 = row("m2")
    nc.vector.tensor_tensor(m2[:], mean[:], mean[:], OP.mult)
    ex2 = row("ex2")
    nc.vector.tensor_scalar(ex2[:], s2[:], 1.0 / DM, None, OP.mult)
    var = row("var")
    nc.vector.tensor_tensor(var[:], ex2[:], m2[:], OP.subtract)
    std = row("std")
    nc.scalar.activation(std[:], var[:], AF.Sqrt, bias=eps_t[:], scale=1.0)
    rstd = row("rstd")
    nc.vector.reciprocal(rstd[:], std[:])
    mrs = row("mrs")
    nc.vector.tensor_tensor(mrs[:], mean[:], rstd[:], OP.mult)
    out = sb.tile([P, DO, T], F32R, tag="ln_out", bufs=4, name=f"{name}_out")
    u = sb.tile([P, DO, T], F32, tag="ln_t", bufs=2, name=f"{name}_u")
    for c in range(DO):
        nc.vector.tensor_tensor(u[:, c, :], xT[:, c, :], rstd[:], OP.mult)
        nc.vector.tensor_tensor(u[:, c, :], u[:, c, :], mrs[:], OP.subtract)
        nc.vector.tensor_scalar(out[:, c, :], u[:, c, :], g_ap[:, c:c + 1],
                                b_ap[:, c:c + 1], OP.mult, OP.add)
    return out


def build_nc():
    nc = bacc.Bacc(None, target_bir_lowering=False, debug=False)

    ein = lambda nm, shp, dt=F32: nc.dram_tensor(nm, shp, dt, kind="ExternalInput")
    x_f = ein("x_f", [1, TL])
    tok_emb = ein("tok_emb", [P, VC, DM], F32R)
    pos_emb = ein("pos_emb", [P, 1, DM], F32R)
    ln_in_g, ln_in_b = ein("ln_in_g", [P, DO]), ein("ln_in_b", [P, DO])
    Wq, Wk, Wv = (ein(n, [L, P, DO, DH], BF16) for n in ("Wq", "Wk", "Wv"))
    Wo = ein("Wo", [L, P, DO, DM], BF16)
    W1 = ein("W1", [L, P, DO, DFF], BF16)
    W2 = ein("W2", [L, P, FO, DM], BF16)
    bq, bk = ein("bq", [L, P, DO]), ein("bk", [L, P, DO])
    bv = ein("bv", [L, 1, DH], F32R)
    bo = ein("bo", [L, P, DO])
    b1, b2 = ein("b1", [L, P, FO]), ein("b2", [L, P, DO])
    ln1_g, ln1_b = ein("ln1_g", [L, P, DO]), ein("ln1_b", [L, P, DO])
    ln2_g, ln2_b = ein("ln2_g", [L, P, DO]), ein("ln2_b", [L, P, DO])
    W_out = ein("W_out", [P, DO, KD], F32R)
    b_out = ein("b_out", [P, DO])
    out = nc.dram_tensor("out", [P, B, KD], F32, kind="ExternalOutput")

    rg = [list(range(NCORES))]

    with tile.TileContext(nc) as tc:
        with tc.tile_pool(name="sb", bufs=1) as sb, \
             tc.tile_pool(name="ps", bufs=1, space="PSUM") as ps, \
             tc.tile_pool(name="dram", bufs=1, space="DRAM") as dram:

            lnp = lambda nm: sb.tile([P, FO], F32, tag="lnp", bufs=24, name=nm)

            ones_f = sb.tile([P, P], F32, tag="ones_f")
            nc.vector.memset(ones_f[:], 1.0)
            ones128_r = sb.tile([P, P], F32R, tag="ones_r")
            nc.vector.tensor_copy(ones128_r[:], ones_f[:])
            ones128_b = sb.tile([P, P], BF16, tag="ones_b")
            nc.vector.tensor_copy(ones128_b[:], ones_f[:])
            ones1_r = sb.tile([1, P], F32R, tag="ones1_r")
            nc.vector.tensor_copy(ones1_r[:], ones_f[0:1, :])
            eps_t = sb.tile([P, 1], F32, tag="eps_t")
            nc.vector.memset(eps_t[:], EPS)

            ident_f = sb.tile([P, P], F32, tag="identf")
            nc.gpsimd.affine_select(out=ident_f[:], in_=ones_f[:],
                                    compare_op=OP.is_equal, fill=0.0,
                                    base=0, channel_multiplier=1, pattern=[[-1, P]])
            ident_rep = sb.tile([P, B * P], F32R, tag="zerom")
            for bb in range(B):
                nc.vector.tensor_copy(ident_rep[:, bb * P:(bb + 1) * P], ident_f[:])

            zero_m = sb.tile([P, 512], F32, tag="zerom")
            nc.gpsimd.memset(zero_m[:], 0.0)
            mask_f = sb.tile([P, 4, 512], BF16, tag="mask")
            for m in range(4):
                nc.gpsimd.affine_select(out=mask_f[:, m, :], in_=zero_m[:],
                                        compare_op=OP.is_ge, fill=NEG,
                                        base=-128 * m, channel_multiplier=-1,
                                        pattern=[[1, 512]])

            iota_i = sb.tile([P, VC], mybir.dt.int32, tag="iota_i")
            nc.gpsimd.iota(iota_i[:], pattern=[[P, VC]], base=0, channel_multiplier=1)
            iota_f = sb.tile([P, VC], F32, tag="iota_f")
            nc.vector.tensor_copy(iota_f[:], iota_i[:])
            xrow = sb.tile([1, TL], F32, tag="rcp", bufs=2)
            nc.sync.dma_start(xrow[:], x_f[:])
            xrow_r = sb.tile([1, TL], F32R, tag="rcp", bufs=2)
            nc.vector.tensor_copy(xrow_r[:], xrow[:])
            xb_ps = ps.tile([P, TL], F32, tag="mm", bufs=3, name="xb_ps")
            nc.tensor.matmul(xb_ps[:], ones1_r[:], xrow_r[:], start=True, stop=True)
            onehot = sb.tile([P, VC, TL], F32R, tag="qk", bufs=2, name="onehot")
            for c in range(VC):
                nc.vector.tensor_scalar(onehot[:, c, :], xb_ps[:], iota_f[:, c:c + 1],
                                        None, OP.is_equal)

            temb = sb.tile([P, VC, DM], F32R, tag="qk", bufs=2, name="temb")
            nc.sync.dma_start(temb[:], tok_emb[:])
            pemb = sb.tile([P, 1, DM], F32R, tag="bvb", bufs=2, name="pemb")
            nc.sync.dma_start(pemb[:], pos_emb[:])
            lnig, lnib = lnp("lnig"), lnp("lnib")
            nc.sync.dma_start(lnig[:, :DO], ln_in_g[:])
            nc.sync.dma_start(lnib[:, :DO], ln_in_b[:])
            h0 = sb.tile([P, DO, TL], F32R, tag="v", bufs=2, name="h0")
            for m in range(DO):
                pe = ps.tile([P, TL], F32, tag="mm", bufs=3, name="pe")
                for c in range(VC):
                    nc.tensor.matmul(pe[:], temb[:, c, m * P:(m + 1) * P],
                                     onehot[:, c, :], start=(c == 0), stop=False)
                nc.tensor.matmul(pe[:], pemb[:, 0, m * P:(m + 1) * P], ident_rep[:],
                                 start=False, stop=True)
                nc.scalar.copy(h0[:, m, :], pe[:])

            h_half = [None, None]
            ag_out = [None, None]

            def emit_ag(hh):
                ag_in = dram.tile([P, DO, TH], BF16, tag="ag_in", bufs=4,
                                  name=f"ag_in{hh}")
                ago = dram.tile([NCORES, P, DO, TH], BF16, tag="ag_out", bufs=4,
                                addr_space="Shared", name=f"ag_out{hh}")
                h16 = sb.tile([P, DO, TH], BF16, tag="h16", bufs=3, name="h16")
                nc.vector.tensor_copy(h16[:], h_half[hh][:])
                nc.sync.dma_start(ag_in[:], h16[:])
                nc.gpsimd.collective_compute(
                    "AllGather", OP.bypass, replica_groups=rg,
                    ins=[ag_in.opt()], outs=[ago.opt()])
                ag_out[hh] = ago

            for hh in range(2):
                h_half[hh] = _ln(nc, sb, ps,
                                 h0[:, :, hh * TH:(hh + 1) * TH],
                                 lnig[:, :DO], lnib[:, :DO], ones128_r, eps_t,
                                 f"ln_in{hh}")
                emit_ag(hh)

            def load_params(l):
                p = {}
                wmat = lambda nm: sb.tile([P, DO, 512], BF16, tag="w1m", bufs=6,
                                          name=nm)
                p["wq"], p["wk"], p["wv"], p["wo"] = (wmat(n) for n in
                                                      ("wq", "wk", "wv", "wo"))
                nc.sync.dma_start(p["wq"][:], Wq[l])
                nc.sync.dma_start(p["wk"][:], Wk[l])
                nc.sync.dma_start(p["wv"][:], Wv[l])
                nc.sync.dma_start(p["wo"][:], Wo[l])
                for nm, t in (("bq", bq), ("bk", bk), ("bo", bo), ("l1g", ln1_g),
                              ("l1b", ln1_b), ("b2", b2), ("l2g", ln2_g),
                              ("l2b", ln2_b)):
                    p[nm] = lnp(nm)
                    nc.sync.dma_start(p[nm][:, :DO], t[l])
                p["b1"] = lnp("b1")
                nc.sync.dma_start(p["b1"][:], b1[l])
                bv_t = sb.tile([1, DH], F32R, tag="bv_t", bufs=2, name="bv_t")
                nc.sync.dma_start(bv_t[:], bv[l])
                pbv = ps.tile([P, DH], F32, tag="mm", bufs=3, name="pbv")
                nc.tensor.matmul(pbv[:], ones1_r[:], bv_t[:], start=True, stop=True)
                p["bvb"] = sb.tile([P, DH], F32, tag="bvb", bufs=2, name="bvb")
                nc.scalar.copy(p["bvb"][:], pbv[:])
                p["rs_in"] = [dram.tile([NCORES, P, DO, 2, P], BF16, tag="rs_in",
                                        bufs=4, name=f"rs_in{hh}") for hh in range(2)]
                p["rs_out"] = [dram.tile([P, DO, 2, P], BF16, tag="rs_out", bufs=4,
                                         name=f"rs_out{hh}") for hh in range(2)]
                p["l"] = l
                return p

            def local_half(p, hh):
                l = p["l"]
                a_loc = sb.tile([P, DO, TH], BF16, tag="h16", bufs=3, name="a_loc")
                nc.sync.dma_start(a_loc[:],
                                  p["rs_out"][hh].rearrange("p o b s -> p o (b s)"))
                x1 = sb.tile([P, DO, TH], F32R, tag="aT", bufs=2, name="x1")
                for m in range(DO):
                    nc.vector.tensor_scalar(x1[:, m, :], a_loc[:, m, :],
                                            p["bo"][:, m:m + 1], None, OP.add)
                nc.vector.tensor_tensor(x1[:], x1[:], h_half[hh][:], OP.add)
                h1 = _ln(nc, sb, ps, x1, p["l1g"][:, :DO], p["l1b"][:, :DO],
                         ones128_r, eps_t, f"ln1_{l}_{hh}")
                h1b = sb.tile([P, DO, TH], BF16, tag="h16", bufs=3, name="h1b")
                nc.vector.tensor_copy(h1b[:], h1[:])
                pz = [ps.tile([P, TH], F32, tag="acc", bufs=4, name=f"pz{m}")
                      for m in range(DO)]
                for qq in range(4):
                    w1q = sb.tile([P, DO, 512], BF16, tag="w1m", bufs=6, name="w1q")
                    nc.sync.dma_start(w1q[:], W1[l, :, :, qq * 512:(qq + 1) * 512])
                    w2q = sb.tile([P, DO, 512], BF16, tag="w1m", bufs=6, name="w2q")
                    nc.sync.dma_start(w2q[:], W2[l, :, qq * DO:(qq + 1) * DO, :])
                    z1q = sb.tile([P, DO, TH], BF16, tag="z1q", bufs=2, name="z1q")
                    for fb in range(DO):
                        f = qq * DO + fb
                        pf = ps.tile([P, TH], F32, tag="mm", bufs=3, name="pf")
                        for c in range(DO):
                            nc.tensor.matmul(pf[:], w1q[:, c, fb * P:(fb + 1) * P],
                                             h1b[:, c, :],
                                             start=(c == 0), stop=(c == DO - 1))
                        nc.scalar.activation(z1q[:, fb, :], pf[:], AF.Relu,
                                             bias=p["b1"][:, f:f + 1], scale=1.0)
                    for m in range(DO):
                        for cc in range(DO):
                            nc.tensor.matmul(pz[m][:], w2q[:, cc, m * P:(m + 1) * P],
                                             z1q[:, cc, :],
                                             start=(qq == 0 and cc == 0),
                                             stop=(qq == 3 and cc == DO - 1))
                z2 = sb.tile([P, DO, TH], F32, tag="aT", bufs=2, name="z2")
                for m in range(DO):
                    nc.scalar.activation(z2[:, m, :], pz[m][:], AF.Identity,
                                         bias=p["b2"][:, m:m + 1], scale=1.0)
                x2 = sb.tile([P, DO, TH], F32R, tag="aT", bufs=2, name="x2")
                nc.vector.tensor_tensor(x2[:], z2[:], h1[:], OP.add)
                h_half[hh] = _ln(nc, sb, ps, x2, p["l2g"][:, :DO], p["l2b"][:, :DO],
                                 ones128_r, eps_t, f"ln2_{l}_{hh}")

            def qkv_attn(p, b):
                hh = b // 2
                hbg = []
                for g in range(NG):
                    t = sb.tile([P, DO, 512], BF16, tag="hbg", bufs=4, name=f"hb{g}")
                    for rr in range(4):
                        r = 4 * g + rr
                        nc.sync.dma_start(
                            t[:, :, rr * P:(rr + 1) * P],
                            ag_out[hh][r, :, :, (b % 2) * P:(b % 2 + 1) * P])
                    hbg.append(t)
                qT = sb.tile([P, DO, S], F32R, tag="qk", bufs=2, name="qT")
                kT = sb.tile([P, DO, S], F32R, tag="qk", bufs=2, name="kT")
                vN = sb.tile([P, S // P, DH], BF16, tag="v", bufs=2, name="vN")
                for m in range(DO):
                    for g in range(NG):
                        sl = slice(g * 512, (g + 1) * 512)
                        pq = ps.tile([P, 512], F32, tag="mm", bufs=3, name="pq")
                        for c in range(DO):
                            nc.tensor.matmul(pq[:], p["wq"][:, c, m * P:(m + 1) * P],
                                             hbg[g][:, c, :],
                                             start=(c == 0), stop=(c == DO - 1))
                        nc.vector.tensor_scalar(qT[:, m, sl], pq[:],
                                                1.0 / math.sqrt(DH),
                                                p["bq"][:, m:m + 1], OP.mult, OP.add)
                        pk = ps.tile([P, 512], F32, tag="mm", bufs=3, name="pk")
                        for c in range(DO):
                            nc.tensor.matmul(pk[:], p["wk"][:, c, m * P:(m + 1) * P],
                                             hbg[g][:, c, :],
                                             start=(c == 0), stop=(c == DO - 1))
                        nc.scalar.activation(kT[:, m, sl], pk[:], AF.Identity,
                                             bias=p["bk"][:, m:m + 1], scale=1.0)
                for tb in range(S // P):
                    pv = ps.tile([P, DH], F32, tag="mm", bufs=3, name="pv")
                    for c in range(DO):
                        nc.tensor.matmul(
                            pv[:], hbg[tb // 4][:, c, (tb % 4) * P:(tb % 4 + 1) * P],
                            p["wv"][:, c, :], start=(c == 0), stop=(c == DO - 1))
                    nc.vector.tensor_tensor(vN[:, tb, :], pv[:], p["bvb"][:], OP.add)

                oT = sb.tile([P, DO, S], BF16, tag="oT", bufs=2, name="oT")
                for g in range(NG):
                    nj = 4 * g + 4
                    sl = slice(g * 512, (g + 1) * 512)
                    attnT = sb.tile([P, S // P, 512], BF16, tag="attnT", bufs=2,
                                    name="attnT")
                    pden = ps.tile([P, 512], F32, tag="ln_ps", bufs=1, name="pden")
                    lo = lambda j: max(0, (j - 4 * g) * P)
                    for j in range(nj):
                        o = lo(j)
                        sc = ps.tile([P, 512], F32, tag="mm", bufs=3, name="sc")
                        for c in range(DO):
                            nc.tensor.matmul(sc[:, o:], kT[:, c, j * P:(j + 1) * P],
                                             qT[:, c, g * 512 + o:(g + 1) * 512],
                                             start=(c == 0), stop=(c == DO - 1))
                        if j >= 4 * g:
                            m = j - 4 * g
                            nc.vector.tensor_tensor(sc[:, o:], sc[:, o:],
                                                    mask_f[:, m, o:], OP.add)
                        nc.scalar.activation(attnT[:, j, o:], sc[:, o:], AF.Exp,
                                             bias=0.0, scale=1.0)
                        nc.tensor.matmul(pden[:, o:], ones128_b[:], attnT[:, j, o:],
                                         start=(j == 0), stop=(j == nj - 1))
                    recip = sb.tile([P, 512], F32, tag="rcp", bufs=2, name="recip")
                    nc.vector.reciprocal_approx_fast(recip[:], pden[:])
                    for d in range(DO):
                        po = ps.tile([P, 512], F32, tag="acc", bufs=4, name="po")
                        for j in range(nj):
                            o = lo(j)
                            nc.tensor.matmul(po[:, o:], vN[:, j, d * P:(d + 1) * P],
                                             attnT[:, j, o:],
                                             start=(j == 0), stop=(j == nj - 1))
                        nc.vector.tensor_tensor(oT[:, d, sl], po[:], recip[:], OP.mult)

                for g in range(NG):
                    sl = slice(g * 512, (g + 1) * 512)
                    aT = sb.tile([P, DO, 512], BF16, tag="a16", bufs=2, name="aT")
                    for m in range(DO):
                        pa = ps.tile([P, 512], F32, tag="mm", bufs=3, name="pa")
                        for c in range(DO):
                            nc.tensor.matmul(pa[:], p["wo"][:, c, m * P:(m + 1) * P],
                                             oT[:, c, sl],
                                             start=(c == 0), stop=(c == DO - 1))
                        nc.scalar.copy(aT[:, m, :], pa[:])
                    for q in range(4):
                        nc.sync.dma_start(p["rs_in"][hh][4 * g + q, :, :, b % 2, :],
                                          aT[:, :, q * P:(q + 1) * P])
                if b % 2 == 1:
                    nc.gpsimd.collective_compute(
                        "ReduceScatter", OP.add, replica_groups=rg,
                        ins=[p["rs_in"][hh].opt()], outs=[p["rs_out"][hh].opt()])

            pcur = load_params(0)
            qkv_attn(pcur, 0)
            qkv_attn(pcur, 1)
            qkv_attn(pcur, 2)
            local_half(pcur, 0)
            emit_ag(0)
            qkv_attn(pcur, 3)
            for l in range(1, L):
                pnext = load_params(l)
                qkv_attn(pnext, 0)
                local_half(pcur, 1)
                emit_ag(1)
                qkv_attn(pnext, 1)
                qkv_attn(pnext, 2)
                local_half(pnext, 0)
                if l < L - 1:
                    emit_ag(0)
                qkv_attn(pnext, 3)
                pcur = pnext
            local_half(pcur, 1)

            wout = sb.tile([P, DO, KD], F32R, tag="oT", bufs=2, name="wout")
            nc.sync.dma_start(wout[:], W_out[:])
            bout = lnp("bout")
            nc.sync.dma_start(bout[:, :DO], b_out[:])
            ident_t = sb.tile([P, P], F32, tag="identf")
            make_identity(nc, ident_t)
            out_sb = sb.tile([P, B, KD], F32, tag="oT", bufs=2, name="out_sb")
            for hh in range(2):
                zT = sb.tile([P, DO, TH], F32, tag="aT", bufs=2, name="zT")
                for m in range(DO):
                    pu = ps.tile([P, TH], F32, tag="mm", bufs=3, name="pu")
                    for c in range(DO):
                        nc.tensor.matmul(pu[:], wout[:, c, m * P:(m + 1) * P],
                                         h_half[hh][:, c, :],
                                         start=(c == 0), stop=(c == DO - 1))
                    nc.scalar.activation(zT[:, m, :], pu[:], AF.Identity,
                                         bias=bout[:, m:m + 1], scale=1.0)
                for kb in range(DO):
                    for tb in range(2):
                        pt = ps.tile([P, P], F32, tag="mm", bufs=3, name="pt")
                        nc.tensor.transpose(pt[:], zT[:, kb, tb * P:(tb + 1) * P],
                                            ident_t[:])
                        nc.scalar.copy(out_sb[:, 2 * hh + tb, kb * P:(kb + 1) * P],
                                       pt[:])
            nc.sync.dma_start(out[:], out_sb[:])

    nc.compile()
    return nc


_NC_CACHE = None


def _get_nc():
    global _NC_CACHE
    if _NC_CACHE is None:
        _NC_CACHE = build_nc()
    return _NC_CACHE


def _lhsT(w):
    Kd, Nd = w.shape
    return np.ascontiguousarray(w.reshape(Kd // P, P, Nd).transpose(1, 0, 2))


def _ppart(v):
    return np.ascontiguousarray(v.reshape(-1, P).T)


def make_in_maps(inputs):
    inp = {k: np.asarray(v) for k, v in inputs.items()}
    x = inp["x"].astype(np.int64)
    f32 = np.float32

    tok_pad = np.zeros((VP, DM), f32)
    tok_pad[:VOCAB] = inp["tok_emb"].astype(f32)
    tok_l = _lhsT(tok_pad)
    stack = lambda fn: np.stack([fn(l) for l in range(L)])

    in_maps = []
    for r in range(NCORES):
        h0c, h1c = r * DH, (r + 1) * DH
        m = {
            "x_f": np.ascontiguousarray(
                x[:, r * P:(r + 1) * P].reshape(-1).astype(f32)[None, :]),
            "tok_emb": tok_l,
            "pos_emb": np.ascontiguousarray(
                inp["pos_emb"][r * P:(r + 1) * P].astype(f32))[:, None, :],
            "ln_in_g": _ppart(inp["ln_in_g"].astype(f32)),
            "ln_in_b": _ppart(inp["ln_in_b"].astype(f32)),
            "Wq": stack(lambda l: _lhsT(inp["Wq"][l, :, h0c:h1c].astype(f32)).astype(BF16NP)),
            "Wk": stack(lambda l: _lhsT(inp["Wk"][l, :, h0c:h1c].astype(f32)).astype(BF16NP)),
            "Wv": stack(lambda l: _lhsT(inp["Wv"][l, :, h0c:h1c].astype(f32)).astype(BF16NP)),
            "Wo": stack(lambda l: _lhsT(inp["Wo"][l, h0c:h1c, :].astype(f32)).astype(BF16NP)),
            "W1": stack(lambda l: _lhsT(inp["W1"][l].astype(f32)).astype(BF16NP)),
            "W2": stack(lambda l: _lhsT(inp["W2"][l].astype(f32)).astype(BF16NP)),
            "bq": stack(lambda l: _ppart(
                inp["bq"][l, h0c:h1c].astype(f32) / np.float32(math.sqrt(DH)))),
            "bk": stack(lambda l: _ppart(inp["bk"][l, h0c:h1c].astype(f32))),
            "bv": np.ascontiguousarray(inp["bv"][:, h0c:h1c].astype(f32)[:, None, :]),
            "bo": stack(lambda l: _ppart(inp["bo"][l].astype(f32))),
            "b1": stack(lambda l: _ppart(inp["b1"][l].astype(f32))),
            "b2": stack(lambda l: _ppart(inp["b2"][l].astype(f32))),
            "ln1_g": stack(lambda l: _ppart(inp["ln1_g"][l].astype(f32))),
            "ln1_b": stack(lambda l: _ppart(inp["ln1_b"][l].astype(f32))),
            "ln2_g": stack(lambda l: _ppart(inp["ln2_g"][l].astype(f32))),
            "ln2_b": stack(lambda l: _ppart(inp["ln2_b"][l].astype(f32))),
            "W_out": _lhsT(inp["W_out"].astype(f32)),
            "b_out": _ppart(inp["b_out"].astype(f32)),
        }
        in_maps.append(m)
    return in_maps


def assemble_output(results):
    full = np.zeros((B, S, KD), np.float32)
    for r in range(NCORES):
        o = results[r]["out"]
        for b in range(B):
            full[b, r * P:(r + 1) * P, :] = o[:, b, :]
    return full


def kernel(**inputs):
    in_maps = make_in_maps(inputs)
    nc = _get_nc()
    res = run_bass_kernel_spmd(nc, in_maps, core_ids=list(range(NCORES)))
    return assemble_output(res.results)
